# revision 1
# baseline (speedup 1.0000x reference)
"""Trainium2 Bass kernel: masked (sparse-adjacency) attention.

Computes, for full inputs:
    adj    = adjs[idx]                      # [Na, N] bool
    scores = (anchor @ wt) @ x.T            # [Na, N]
    atten  = softmax(where(adj, scores, -inf) / T, axis=1)
    out    = weight[idx] * (atten @ x)      # [Na, d_out]

Sharding: anchors split across 8 cores, 1280 per core (Na padded to
10240). x / wt replicated; adjacency shipped pre-transposed per shard.

v3 design (per core). Scores are computed in the log2 domain: wt is
pre-scaled by 1/(T*ln2) on the host so the S-matmul yields
z = s/(T*ln2) and exp(s/T) == 2^z.

The K=64 S-matmuls only use half the 128-row PE array, so consecutive
j-tiles are ROW-TILED onto the two halves and run concurrently (even j
on array rows 0-63 via partitions 0-63, odd j on rows 64-127 via
tile_position=(64,0)), which halves S-matmul streaming time. x^T and
q^T are laid out twice (top/bottom partition halves) to feed the two
row groups.

Per j-tile of 128 x-rows (79 tiles, processed in 40 pairs):
  - S^T_A[j] = xT_j.T @ qt   PE f32r, anchors 0:1024, row-tiled pair
  - S^T_D[j] = xT_j.T @ qt   PE f32r, anchors 1024:1280 (both j of a
               pair share one PSUM bank, sequential)
  - p_A = exp(ln2*z_A)       ACT spline exp, PSUM->SBUF bf16
  - p_D = fastexp(z_D)       DVE: i16 = round(128*z + B7); the int16
               bits ARE bf16(2^z) (Schraudolph, ~±3%/elem, cancels in
               the softmax ratio; 256/1280 anchors only)
  - pm  = p * adjT           DVE bf16 2x (1024-col + 256-col muls)
  - O  += [X | 1].T @ pm     PE bf16, accumulated over j; the ones
               column yields softmax denominators
  O-matmuls are emitted 2 j-iterations late (software pipelining) so
  the in-order PE queue never head-of-line blocks on the ACT/DVE
  chain.
  PSUM banks: sA pair 2x2 + sD pair 1 + oA 2 + oD 1 = 8/8.
  DMA: one contiguous 320KB adjacency DMA per j on the SP ring; all
  prologue loads ride the Activation-engine HWDGE ring so the
  adjacency stream starts at t=0.
Tail: PE-transpose O^T back to [a, 65], scale rows by
weight[idx] / denom, DMA out.
"""

import numpy as np
import ml_dtypes

import concourse.bacc as bacc
import concourse.bass as bass
import concourse.mybir as mybir
import concourse.tile as tile
from concourse.bass_utils import run_bass_kernel_spmd

F32 = mybir.dt.float32
F32R = mybir.dt.float32r  # fp32 fast-path: 1 PE cycle/row at N>=256
BF16 = mybir.dt.bfloat16
I16 = mybir.dt.int16

N_CORES = 8
N = 10000          # x rows (softmax width)
NA = 10000         # anchors
D_IN = 256
D_OUT = 64
TEMP = 0.07
LN2 = float(np.log(2.0))

NJ_TILES = 79                 # ceil(10000 / 128)
NJ = NJ_TILES * 128           # 10112, padded x-rows
NPAIR = 40                    # j-tile pairs (last pair has one j)
A_CORE = 1280                 # anchors per core (10240 padded / 8)
W_A = 1024                    # anchor cols exp'd on ACT (spline, exact)
W_D = 256                     # anchor cols exp'd on DVE (fast-exp bits)
B7 = 127.0 * 128.0 - 5.5      # fast-exp additive constant (bf16 bits)
M_AUG = D_OUT + 1             # 65: d_out columns + ones column


def _build_bass():
    nc = bacc.Bacc(
        "TRN2",
        target_bir_lowering=False,
        debug=False,
        num_devices=N_CORES,
    )
    xT = nc.dram_tensor("xT", [D_OUT, NJ], F32R, kind="ExternalInput").ap()
    xaug = nc.dram_tensor(
        "xaug", [128, NJ_TILES * M_AUG], BF16, kind="ExternalInput"
    ).ap()
    anchT = nc.dram_tensor("anchT", [D_IN, A_CORE], F32R, kind="ExternalInput").ap()
    adjT = nc.dram_tensor("adjT", [NJ, A_CORE], BF16, kind="ExternalInput").ap()
    wt = nc.dram_tensor("wt", [D_IN, D_OUT], F32R, kind="ExternalInput").ap()
    wscale = nc.dram_tensor("wscale", [128, 1], F32, kind="ExternalInput").ap()
    ident = nc.dram_tensor("ident", [128, 128], F32, kind="ExternalInput").ap()
    out = nc.dram_tensor("out", [A_CORE, D_OUT], F32, kind="ExternalOutput").ap()

    EXP = mybir.ActivationFunctionType.Exp
    MULT = mybir.AluOpType.mult
    ADD = mybir.AluOpType.add

    with tile.TileContext(nc) as tc:
        with tc.tile_pool(name="const", bufs=1) as const:
            wt0t = const.tile([128, D_OUT], F32R)
            nc.scalar.dma_start(wt0t[:], wt[0:128, :])
            wt1t = const.tile([128, D_OUT], F32R)
            nc.scalar.dma_start(wt1t[:], wt[128:256, :])
            an0t = const.tile([128, A_CORE], F32R)
            nc.scalar.dma_start(an0t[:], anchT[0:128, :])
            an1t = const.tile([128, A_CORE], F32R)
            nc.scalar.dma_start(an1t[:], anchT[128:256, :])
            wt0 = wt0t[:]
            wt1 = wt1t[:]
            an0 = an0t[:]
            an1 = an1t[:]
            # x / tail constants ride the SP ring (ahead of the
            # adjacency stream, which has slack) so they drain in
            # parallel with the q^T operands on the Activation ring
            xT_sb = const.tile([D_OUT, NJ], F32R)
            for c0 in range(0, NJ, 2528):
                nc.sync.dma_start(
                    xT_sb[:, c0 : c0 + 2528], xT[:, c0 : c0 + 2528]
                )
            xaug_sb = const.tile([128, NJ_TILES * M_AUG], BF16)
            half = 40 * M_AUG
            nc.sync.dma_start(xaug_sb[:, 0:half], xaug[:, 0:half])
            nc.sync.dma_start(xaug_sb[:, half:], xaug[:, half:])
            ident_sb = const.tile([128, 128], F32)
            nc.sync.dma_start(ident_sb[:], ident[:])
            wscale_sb = const.tile([128, 1], F32)
            nc.sync.dma_start(wscale_sb[:], wscale[:])
            qt_sb = const.tile([D_OUT, A_CORE], F32R)
            ot_sb = const.tile([M_AUG, A_CORE], F32)

            # ---- Q^T = wt_s.T @ anchor.T -> [64, 1280] ----
            with tc.tile_pool(name="prepsum", bufs=1, space="PSUM") as prepsum:
                qt_ps = prepsum.tile([D_OUT, A_CORE], F32, padded_shape=[D_OUT, 1536])
                for n0 in range(0, A_CORE, 512):
                    nw = min(512, A_CORE - n0)
                    nc.tensor.matmul(
                        qt_ps[:, n0 : n0 + nw],
                        wt0,
                        an0[:, n0 : n0 + nw],
                        start=True,
                        stop=False,
                    )
                    nc.tensor.matmul(
                        qt_ps[:, n0 : n0 + nw],
                        wt1,
                        an1[:, n0 : n0 + nw],
                        start=False,
                        stop=True,
                    )
                nc.vector.tensor_copy(qt_sb[:], qt_ps[:])
                # PE warm-up: keep the tensor engine streaming through
                # the xT/adjacency DMA wait so the HAM clock-gate stays
                # at 2.4 GHz into the main loop (operands are already
                # resident, so these never stall the queue)
                for _ in range(28):
                    nc.tensor.matmul(
                        qt_ps[:, 1024:1280],
                        wt0,
                        an0[:, 0:256],
                        start=True,
                        stop=True,
                    )

            # ---- main loop over j-pairs, O-matmuls 2 j's behind ----
            with (
                tc.tile_pool(name="adjp", bufs=6) as adjp,
                tc.tile_pool(name="pp", bufs=3) as pp,
                tc.tile_pool(name="pmp", bufs=4) as pmp,
                tc.tile_pool(name="spsumA", bufs=2, space="PSUM") as spsumA,
                tc.tile_pool(name="spsumD", bufs=1, space="PSUM") as spsumD,
                tc.tile_pool(name="opsumA", bufs=1, space="PSUM") as opsumA,
                tc.tile_pool(name="opsumD", bufs=1, space="PSUM") as opsumD,
            ):
                oA = opsumA.tile([M_AUG, W_A], F32)
                oD = opsumD.tile([M_AUG, W_D], F32, padded_shape=[M_AUG, 512])
                pms = [None] * NJ_TILES

                def emit_o(j):
                    xa_w = xaug_sb[:, j * M_AUG : (j + 1) * M_AUG]
                    pm_t = pms[j]
                    pms[j] = None
                    for n0 in (0, 512):
                        nc.tensor.matmul(
                            oA[:, n0 : n0 + 512],
                            xa_w,
                            pm_t[:, n0 : n0 + 512],
                            start=(j == 0),
                            stop=(j == NJ_TILES - 1),
                        )
                    nc.tensor.matmul(
                        oD[:],
                        xa_w,
                        pm_t[:, W_A:A_CORE],
                        start=(j == 0),
                        stop=(j == NJ_TILES - 1),
                    )

                for p in range(NPAIR):
                    js = [2 * p + jj for jj in (0, 1) if 2 * p + jj < NJ_TILES]
                    for j in js:
                        adj_t = adjp.tile([128, A_CORE], BF16)
                        nc.sync.dma_start(
                            adj_t[:], adjT[j * 128 : (j + 1) * 128, :]
                        )
                        pms[j] = (adj_t, None, None, None)
                    # S-matmuls: A-chunk row-tiled pair (concurrent on
                    # the two array halves), then D-chunk sequential
                    # into one shared PSUM bank
                    # D-chunk first: its pair-tile dependency cleared a
                    # full pair ago, so these fill the PE queue while the
                    # A-chunk PSUM-free semaphores are still in flight
                    sDp = spsumD.tile([128, 2 * W_D], F32)
                    for j in js:
                        xt_w = xT_sb[:, j * 128 : (j + 1) * 128]
                        c = W_D * (j & 1)
                        nc.tensor.matmul(
                            sDp[:, c : c + W_D],
                            xt_w,
                            qt_sb[:, W_A:A_CORE],
                            start=True,
                            stop=True,
                        )
                    sAs = []
                    for j in js:
                        xt_w = xT_sb[:, j * 128 : (j + 1) * 128]
                        sA = spsumA.tile([128, W_A], F32)
                        for n0 in (0, 512):
                            nc.tensor.matmul(
                                sA[:, n0 : n0 + 512],
                                xt_w,
                                qt_sb[:, n0 : n0 + 512],
                                start=True,
                                stop=True,
                            )
                        sAs.append(sA)
                    # exp + mask, then O-matmuls two j's behind
                    for i, j in enumerate(js):
                        adj_t = pms[j][0]
                        p_t = pp.tile([128, A_CORE], BF16)
                        nc.scalar.activation(
                            p_t[:, 0:W_A], sAs[i][:], EXP, scale=LN2
                        )
                        c = W_D * (j & 1)
                        nc.vector.tensor_scalar(
                            p_t[:, W_A:A_CORE].bitcast(I16),
                            sDp[:, c : c + W_D],
                            128.0,
                            B7,
                            MULT,
                            ADD,
                        )
                        pm_t = pmp.tile([128, A_CORE], BF16)
                        nc.vector.tensor_mul(pm_t[:], p_t[:], adj_t[:])
                        pms[j] = pm_t
                        if j >= 2:
                            emit_o(j - 2)
                emit_o(NJ_TILES - 2)
                emit_o(NJ_TILES - 1)
                nc.vector.tensor_copy(ot_sb[:, 0:W_A], oA[:])
                nc.vector.tensor_copy(ot_sb[:, W_A:A_CORE], oD[:])

            # ---- tail: transpose back, normalize, scale, store ----
            with (
                tc.tile_pool(name="tpsum", bufs=4, space="PSUM") as tpsum,
                tc.tile_pool(name="tail", bufs=4) as tail,
            ):
                for k in range(A_CORE // 128):
                    t_ps = tpsum.tile([128, M_AUG], F32)
                    nc.tensor.transpose(
                        t_ps[:],
                        ot_sb[0:M_AUG, k * 128 : (k + 1) * 128],
                        ident_sb[0:M_AUG, 0:M_AUG],
                    )
                    rec = tail.tile([128, 1], F32)
                    nc.vector.reciprocal(rec[:], t_ps[:, D_OUT : D_OUT + 1])
                    rec2 = tail.tile([128, 1], F32)
                    nc.vector.tensor_mul(rec2[:], rec[:], wscale_sb[:])
                    o_t = tail.tile([128, D_OUT], F32)
                    nc.vector.tensor_scalar_mul(o_t[:], t_ps[:, 0:D_OUT], rec2[:])
                    nc.sync.dma_start(out[k * 128 : (k + 1) * 128, :], o_t[:])

    nc.compile()
    return nc


def _prep_inputs(x, weight, adjs, idx, anchor, wt):
    i = int(np.asarray(idx))
    x = np.asarray(x, dtype=np.float32)
    anchor = np.asarray(anchor, dtype=np.float32)
    wt = np.asarray(wt, dtype=np.float32)
    adj = np.asarray(adjs)[i]  # [Na, N] bool
    w = float(np.asarray(weight)[i])

    NAP = N_CORES * A_CORE  # 10240

    xT = np.zeros((D_OUT, NJ), dtype=np.float32)
    xT[:, :N] = x.T

    xaug = np.zeros((NJ, M_AUG), dtype=ml_dtypes.bfloat16)
    xaug[:N, :D_OUT] = x
    xaug[:N, D_OUT] = 1.0
    xaug_strip = np.ascontiguousarray(
        xaug.reshape(NJ_TILES, 128, M_AUG).transpose(1, 0, 2).reshape(128, -1)
    )

    anchorT = np.zeros((D_IN, NAP), dtype=np.float32)
    anchorT[:, :NA] = anchor.T

    # wt pre-scaled so the S-matmul lands in the log2 domain
    wt_s = np.ascontiguousarray(wt * np.float32(1.0 / (TEMP * LN2)))

    # adjacency, transposed to [N, Na], as bf16 {0.0, 1.0}
    adj_u16 = np.zeros((NJ, NAP), dtype=np.uint16)
    adj_u16[:N, :NA] = adj.T
    adj_u16 *= 0x3F80  # bf16 bit pattern of 1.0
    # padded anchor columns: one fake edge to x-row 0 so denominators
    # are finite (those rows are discarded on the host)
    adj_u16[0, NA:] = 0x3F80
    adj_bf = adj_u16.view(ml_dtypes.bfloat16)

    ident = np.eye(128, dtype=np.float32)
    wscale = np.full((128, 1), w, dtype=np.float32)

    in_maps = []
    for c in range(N_CORES):
        sl = slice(c * A_CORE, (c + 1) * A_CORE)
        in_maps.append(
            {
                "xT": xT,
                "xaug": xaug_strip,
                "anchT": np.ascontiguousarray(anchorT[:, sl]),
                "adjT": np.ascontiguousarray(adj_bf[:, sl]),
                "wt": wt_s,
                "wscale": wscale,
                "ident": ident,
            }
        )
    return in_maps


def run(x, weight, adjs, idx, anchor, wt, trace=False, **spmd_kwargs):
    in_maps = _prep_inputs(x, weight, adjs, idx, anchor, wt)
    nc = _build_bass()
    res = run_bass_kernel_spmd(
        nc, in_maps, core_ids=list(range(N_CORES)), trace=trace, **spmd_kwargs
    )
    out = np.concatenate([r["out"] for r in res.results], axis=0)[:NA]
    return np.ascontiguousarray(out.astype(np.float32)), res


def kernel(x, weight, adjs, idx, anchor, wt):
    out, _ = run(x, weight, adjs, idx, anchor, wt)
    return out

